# revision 45
# baseline (speedup 1.0000x reference)
"""AdaAggLayer Trainium2 kernel.

Data-parallel over batch: 8 NeuronCores x 4 samples each. Per core:
  - attention (global avg pool -> 1x1 -> relu -> 1x1 -> sigmoid) on PE/ACT/DVE
  - align transform w_alT[e] = (align[e] @ w[e]) stored transposed [i,o] on PE
  - per-sample weight aggregation sum_e att[b,e]*w_alT[e] on DVE (bf16)
  - per-sample 3x3 conv as 9 shifted matmuls accumulating in PSUM (bf16)
  - bias epilogue fused into the PSUM->SBUF copy on ACT
No collectives: inputs are sharded/replicated host-side, outputs concatenated.
"""

import contextlib
import importlib.util
import os
import sys
import types

sys.path.insert(0, "/opt/trn_rl_repo")

import numpy as np
import ml_dtypes

import concourse.bass as bass
import concourse.mybir as mybir
import concourse.tile as tile
from concourse import bacc
from concourse.bass_utils import run_bass_kernel_spmd

N_CORES = 8
B, I, O, E, HID = 32, 256, 256, 5, 65
H = W = 56
HP = H + 2  # zero-padded spatial
BL = B // N_CORES  # samples per core
KK = 9  # 3x3 taps
NBLK = 7  # row blocks of 8 output rows
RB = 8  # rows per block
BF16 = mybir.dt.bfloat16
F32 = mybir.dt.float32

_NC_CACHE = None


def _install_ntff_hook():
    """Register the axon NTFF profiling hook (the image's antenv lacks it)."""
    if "antenv.axon_hooks" in sys.modules:
        return
    try:
        spec = importlib.util.spec_from_file_location(
            "trn_boot", "/root/.axon_site/trn_agent_boot/trn_boot.py"
        )
        tb = importlib.util.module_from_spec(spec)
        spec.loader.exec_module(tb)
        hook = tb._ntff_profile_via_ctypes("/opt/axon/libaxon_pjrt.so")
    except Exception:
        hook = None
    mod = types.ModuleType("antenv.axon_hooks")
    mod.get_axon_ntff_profile_hook = lambda: hook
    sys.modules["antenv.axon_hooks"] = mod


def _emit(nc, tc, ctx):
    x_d = nc.dram_tensor("x", [BL, I, HP, HP], BF16, kind="ExternalInput")
    w_d = nc.dram_tensor("w", [E, O, I * KK], BF16, kind="ExternalInput")
    at_d = nc.dram_tensor("alignT", [E, O, O], BF16, kind="ExternalInput")
    w1_d = nc.dram_tensor("w1T", [I, HID], F32, kind="ExternalInput")
    # attn_w2.T with attn_b2 appended as a trailing row; paired with a
    # constant-1 row in h so the second 1x1 conv's bias rides the matmul.
    w2_d = nc.dram_tensor("w2Ta", [HID + 1, E], F32, kind="ExternalInput")
    bias_d = nc.dram_tensor("bias", [E, O], F32, kind="ExternalInput")
    out_d = nc.dram_tensor("out", [BL, O, H, W], F32, kind="ExternalOutput")

    const = ctx.enter_context(tc.tile_pool(name="const", bufs=1))
    wstream = ctx.enter_context(tc.tile_pool(name="wstream", bufs=6))
    xpool = ctx.enter_context(tc.tile_pool(name="x", bufs=1))
    aggp = ctx.enter_context(tc.tile_pool(name="agg", bufs=BL))
    tmpp = ctx.enter_context(tc.tile_pool(name="tmp", bufs=2))
    stagep = ctx.enter_context(tc.tile_pool(name="stage", bufs=4))
    t_psum = ctx.enter_context(tc.tile_pool(name="tps", bufs=2, space="PSUM"))
    s_psum = ctx.enter_context(tc.tile_pool(name="sps", bufs=1, space="PSUM"))
    c_psum = ctx.enter_context(tc.tile_pool(name="cps", bufs=5, space="PSUM"))
    NG = 3  # kk chunks per aggregation group (3 groups of 3)

    # ---- constants in ----
    at_sb = const.tile([128, E, 2, O], BF16)  # part = o_old % 128
    w1_sb = const.tile([128, 2, HID], F32)  # part = i % 128
    w2_sb = const.tile([HID + 1, E], F32)
    bias_sb = const.tile([E, O], F32)
    ones_sb = const.tile([1, 128], F32)
    walT = const.tile([128, E, KK, 2, O], BF16)  # part = i % 128 (per i-half)
    pooledT = const.tile([128, 2, BL], F32)  # part = i % 128
    h_sb = const.tile([HID + 1, BL], F32)  # row HID is constant 1.0
    att_sb = const.tile([E, BL], F32)
    att_row = const.tile([1, BL * E], F32)
    att_bc = const.tile([128, BL, E], F32)
    aggb_sb = const.tile([128, 2, BL], F32)  # part = o % 128

    x_sb = {}

    def dma_x(b):
        for ih in range(2):
            t = xpool.tile([128, HP, HP], BF16, tag=f"x{b}_{ih}")
            nc.sync.dma_start(out=t[:, :, :], in_=x_d[b, ih * 128 : (ih + 1) * 128, :, :])
            x_sb[(b, ih)] = t

    def attention(bs):
        # batched attention for the samples in bs (contiguous range).
        b0, nb = bs[0], len(bs)
        # pooled sums on DVE (1x, ~3.5us per half)
        for b in bs:
            for ih in range(2):
                nc.vector.reduce_sum(
                    out=pooledT[:, ih, b : b + 1],
                    in_=x_sb[(b, ih)][:, :, :],
                    axis=mybir.AxisListType.XY,
                )
        hp = s_psum.tile([HID, BL], F32, tag="sps")
        for ih in range(2):
            nc.tensor.matmul(
                hp[:, :nb],
                lhsT=w1_sb[:, ih, :],
                rhs=pooledT[:, ih, b0 : b0 + nb],
                start=(ih == 0),
                stop=(ih == 1),
            )
        nc.scalar.activation(
            h_sb[:HID, b0 : b0 + nb], hp[:, :nb], mybir.ActivationFunctionType.Relu
        )
        # att columns [e, b] for the aggregated-bias matmul
        ap = s_psum.tile([E, BL], F32, tag="sps")
        nc.tensor.matmul(ap[:, :nb], lhsT=w2_sb[:, :], rhs=h_sb[:, b0 : b0 + nb])
        nc.scalar.activation(
            att_sb[:, b0 : b0 + nb], ap[:, :nb], mybir.ActivationFunctionType.Sigmoid
        )
        # att row per sample on partition 0 (M=1 matmul), then one broadcast
        # matmul to all 128 partitions -- no SBUF-to-SBUF DMA involved.
        rp = s_psum.tile([1, BL * E], F32, tag="sps")
        for j, b in enumerate(bs):
            nc.tensor.matmul(
                rp[0:1, j * E : (j + 1) * E],
                lhsT=h_sb[:, b : b + 1],
                rhs=w2_sb[:, :],
            )
        nc.scalar.activation(
            att_row[0:1, b0 * E : (b0 + nb) * E],
            rp[0:1, : nb * E],
            mybir.ActivationFunctionType.Sigmoid,
        )
        bp = s_psum.tile([128, BL * E], F32, tag="sps")
        nc.tensor.matmul(
            bp[:, : nb * E],
            lhsT=ones_sb[0:1, :],
            rhs=att_row[0:1, b0 * E : (b0 + nb) * E],
        )
        nc.vector.tensor_copy(
            out=att_bc[:, b0 : b0 + nb, :], in_=bp[:, : nb * E]
        )
        # aggregated bias agg_b[o, b] = sum_e att[e,b] * bias[e, o]
        for ot in range(2):
            gp = s_psum.tile([128, BL], F32, tag="sps")
            nc.tensor.matmul(
                gp[:, :nb],
                lhsT=bias_sb[:, ot * 128 : (ot + 1) * 128],
                rhs=att_sb[:, b0 : b0 + nb],
            )
            nc.vector.tensor_copy(out=aggb_sb[:, ot, b0 : b0 + nb], in_=gp[:, :nb])

    # per-sample aggregation chunk: scale+add tree on DVE --
    # tensor_scalar (4x bf16) + tensor_tensor (2x bf16) beats the 1x-mode
    # fused scalar_tensor_tensor chain. Chunked by kk group so chunks
    # pipeline with the transform / conv.
    aggs_all = {}

    def agg_chunk(b, g):
        k0, k1 = g * 3, g * 3 + 3
        agg = aggp.tile([128, 3, 2, O], BF16, tag=f"agg{g}")
        nc.vector.tensor_scalar_mul(
            agg[:, :, :, :], walT[:, 0, k0:k1, :, :], att_bc[:, b, 0:1]
        )
        for e in range(1, E):
            tmp = tmpp.tile([128, 3, 2, O], BF16, tag="tmp")
            nc.vector.tensor_scalar_mul(
                tmp[:, :, :, :], walT[:, e, k0:k1, :, :], att_bc[:, b, e : e + 1]
            )
            nc.vector.tensor_add(
                out=agg[:, :, :, :], in0=agg[:, :, :, :], in1=tmp[:, :, :, :]
            )
        aggs_all[(b, g)] = agg

    # w(e=0) first (transform head), then x0 (attention-b0 head), then the rest
    wt_all = {}

    def dma_w(e):
        for oh in range(2):
            t = wstream.tile([128, I, KK], BF16, tag="wst")
            nc.sync.dma_start(out=t[:, :, :], in_=w_d[e, oh * 128 : (oh + 1) * 128, :])
            wt_all[(e, oh)] = t
            nc.sync.dma_start(
                out=at_sb[:, e, oh, :], in_=at_d[e, oh * 128 : (oh + 1) * 128, :]
            )

    dma_w(0)
    dma_w(1)
    dma_x(0)
    for e in range(2, E):
        dma_w(e)
    for ih in range(2):
        nc.sync.dma_start(out=w1_sb[:, ih, :], in_=w1_d[ih * 128 : (ih + 1) * 128, :])
    nc.sync.dma_start(out=w2_sb[:, :], in_=w2_d[:, :])
    nc.sync.dma_start(out=bias_sb[:, :], in_=bias_d[:, :])
    nc.vector.memset(ones_sb[:, :], 1.0)
    # partition starts must be 32-aligned: memset rows 64-65, relu later
    # overwrites row 64 with real h values; row 65 stays the constant 1.0.
    nc.vector.memset(h_sb[HID - 1 : HID + 1, :], 1.0)
    for b in range(1, BL):
        dma_x(b)

    # ---- align transform: walT[e][kk, ih][ii, o] = sum_o_old w[e,o_old,(i,kk)] alignT[e][o_old, o]
    # Full-bank PSUM groups: both ih halves of one (e,kk) in one [128,512] bank.
    # Evacuations alternate DVE/ACT so neither engine backpressures the PE.
    # e-major order (keeps the w DMA stream pipelined with the transform).
    # attention(0) is slotted after e=1; agg0's chunks are slotted inside
    # the e=4 section right after their last (e4, kk) dependency, so agg0
    # is ready when the transform's PE work drains and conv0 starts cold-free.
    for e in range(E):
        for kk in range(KK):
            tp = t_psum.tile([128, 2, O], F32, tag="tps")
            for ih in range(2):
                for oh in range(2):
                    nc.tensor.matmul(
                        tp[:, ih, :],
                        lhsT=wt_all[(e, oh)][:, ih * 128 : (ih + 1) * 128, kk],
                        rhs=at_sb[:, e, oh, :],
                        start=(oh == 0),
                        stop=(oh == 1),
                    )
            # e<4: alternate DVE/ACT; e==4: all ACT so DVE is free to run
            # agg0's chunks as soon as their walT inputs land.
            if e != E - 1 and (e * KK + kk) % 2 == 0:
                nc.vector.tensor_copy(out=walT[:, e, kk, :, :], in_=tp[:, :, :])
            else:
                nc.scalar.activation(
                    walT[:, e, kk, :, :],
                    tp[:, :, :],
                    mybir.ActivationFunctionType.Copy,
                )
            if e == E - 1 and kk % 3 == 2:
                agg_chunk(0, kk // 3)
        if e == 1:
            attention([0])

    # ---- per-sample: aggregate weights (DVE, kk-chunked) then conv (PE) ----
    for b in range(BL):
        for g in range(NG):
            if (b, g) not in aggs_all:
                agg_chunk(b, g)
        aggs = [aggs_all[(b, g)] for g in range(NG)]

        # block-groups of 4/3 row-blocks: the (g,kq,ih) weight loads are
        # shared across the group's interleaved PSUM banks, amortizing
        # LDWEIGHTS 4x (PE accumulates to different banks between loads).
        for ot in range(2):
            for blks in ([0, 1, 2, 3], [4, 5, 6]):
                cps = {
                    blk: c_psum.tile(
                        [128, RB, W], F32, tag="cps", name=f"cp{b}_{ot}_{blk}"
                    )
                    for blk in blks
                }
                for g in range(NG):
                    for kq in range(3):
                        kk = g * 3 + kq
                        di, dj = kk // 3 - 1, kk % 3 - 1
                        for ih in range(2):
                            for blk in blks:
                                r0 = blk * RB
                                nc.tensor.matmul(
                                    cps[blk][:, :, :],
                                    lhsT=aggs[g][:, kq, ih, ot * 128 : (ot + 1) * 128],
                                    rhs=x_sb[(b, ih)][
                                        :,
                                        r0 + di + 1 : r0 + di + 1 + RB,
                                        dj + 1 : dj + 1 + W,
                                    ],
                                    start=(g == 0 and kq == 0 and ih == 0),
                                    stop=(g == NG - 1 and kq == 2 and ih == 1),
                                )
                for blk in blks:
                    r0 = blk * RB
                    st = stagep.tile([128, RB, W], F32, tag="stage")
                    nc.scalar.activation(
                        st[:, :, :],
                        cps[blk][:, :, :],
                        mybir.ActivationFunctionType.Identity,
                        bias=aggb_sb[:, ot, b : b + 1],
                    )
                    nc.sync.dma_start(
                        out=out_d[b, ot * 128 : (ot + 1) * 128, r0 : r0 + RB, :],
                        in_=st[:, :, :],
                    )
                # remaining samples' attention rides inside conv0's stream
                # (second half: by then the pooled reduces have drained on
                # DVE); it finishes long before conv1 needs agg1.
                if b == 0 and ot == 1 and blks[0] == 0:
                    attention([1, 2, 3])


def _build():
    nc = bacc.Bacc("TRN2", target_bir_lowering=False, debug=False, num_devices=N_CORES)
    with contextlib.ExitStack() as ctx:
        tc = ctx.enter_context(tile.TileContext(nc))
        _emit(nc, tc, ctx)
    nc.compile()
    return nc


def _get_nc():
    global _NC_CACHE
    if _NC_CACHE is None:
        _NC_CACHE = _build()
    return _NC_CACHE


def _run(trace=False, **inputs):
    x = np.asarray(inputs["x"], np.float32)
    weight = np.asarray(inputs["weight"], np.float32)
    bias = np.asarray(inputs["bias"], np.float32)
    align = np.asarray(inputs["align"], np.float32)
    w1 = np.asarray(inputs["attn_w1"], np.float32)
    w2 = np.asarray(inputs["attn_w2"], np.float32)
    b2 = np.asarray(inputs["attn_b2"], np.float32)

    xp = np.zeros((B, I, HP, HP), dtype=ml_dtypes.bfloat16)
    xp[:, :, 1 : 1 + H, 1 : 1 + W] = x
    w_bf = weight.reshape(E, O, I * KK).astype(ml_dtypes.bfloat16)
    alT = np.ascontiguousarray(align.transpose(0, 2, 1)).astype(ml_dtypes.bfloat16)
    w1T = np.ascontiguousarray((w1 / float(H * W)).T)
    w2Ta = np.ascontiguousarray(
        np.concatenate([w2.T, b2.reshape(1, E)], axis=0)
    ).astype(np.float32)

    nc = _get_nc()
    in_maps = []
    for c in range(N_CORES):
        in_maps.append(
            {
                "x": xp[c * BL : (c + 1) * BL],
                "w": w_bf,
                "alignT": alT,
                "w1T": w1T,
                "w2Ta": w2Ta,
                "bias": bias,
            }
        )
    if trace:
        _install_ntff_hook()
    res = run_bass_kernel_spmd(
        nc, in_maps, core_ids=list(range(N_CORES)), trace=trace
    )
    out = np.concatenate([res.results[c]["out"] for c in range(N_CORES)], axis=0)
    return out, res


def kernel(**inputs):
    out, _ = _run(trace=False, **inputs)
    return out


def kernel_profiled(**inputs):
    out, res = _run(trace=True, **inputs)
    return out, res
